# revision 50
# baseline (speedup 1.0000x reference)
"""Butterworth bandpass (cascaded biquad IIR) Trainium2 kernel.

Problem: y = sosfilt(sos, x) over x[32, 64, 4096] fp32 -- 2048 independent
signals, 4 cascaded DF2T biquads, sequential over T=4096.

Strategy (exact block-parallel reformulation):
  The cascade is a linear state-space system (A[8,8], B, C, D).  Split T
  into blocks of L=120 steps, two blocks per window (W=240).  The input is
  pre-transposed and fp16-packed on the HOST into [tau, block, signal]
  layout, so no PE transposes are needed, and the 8-dim state s_w at each
  window entry is carried in the 8 spare partition rows (120..127) of the
  block-0 operand tile.  All filter operators are folded into two fp16
  tables built on host in float64:

    T0[128, 240]: rows 0..119 = [Th | (Z F)^T]   (conv + cross-block)
                  rows 120..  = [Z^T | (Z A_L)^T] (entry-state correction)
    G0[128, 8]:   rows 0..119 = (A_L F)^T, rows 120.. = (A_L^2)^T
    G1[120, 8]:   F^T

  Per window only 3 matmuls (all operands fp16, psum fp32), computing the
  TRANSPOSED output yT[t, sig] so the coefficient tables are the stationary
  operand and each matmul streams all 256 signals (N=256):
    psum[128, 512]: cols 0:256 = block0 outputs, cols 256:512 = block1
    mm3: psum[:,256:512]  = L3.T @ b1   (block1 conv + F x1 state part)
    mm1: psum[:,0:256]    = L1.T @ b0   (block0 conv + Z s correction)
    mm2: psum[:,256:512] += L2.T @ b0   (cross-block + Z A_L s + state upd)
  The 8-dim state rides in 8 spare K-rows of b0 and 8 spare output rows of
  the block1 columns, so corrections and the state update are free.  mm3 is
  state-independent and ordered first: it fills the PE while the previous
  window's state copy is in flight.  Engine accesses must start at a
  32-aligned partition, so state rows live at partitions 96..103 (input
  taus 96..119 shift to rows 104..127); tables are permuted to match in
  rows (K) and columns (psum partitions).  y returns to HBM fp16 in the
  same packed [row, block*256+sig] layout as the input (un-packed on the
  host); 2048 signals are sharded 256 per NeuronCore.
"""

import numpy as np

import concourse.bass as bass
import concourse.tile as tile
from concourse import bacc
from concourse import mybir
from concourse.bass_utils import run_bass_kernel_spmd

FP32 = mybir.dt.float32
FP16 = mybir.dt.float16

P = 128            # partition width
L = 120            # time-block length (128 - 8 state rows)
SROW = 96          # partition row where the 8 state rows live (32-aligned)
NST = 8            # state dim of the 4-biquad cascade
R = 2              # blocks per window
W = R * L          # 240 time steps per window
T = 4096
NWIN = 18          # 18 windows cover 4320 >= 4096 (last window: 16 real steps)
TPAD = NWIN * W    # 4320
NCORES = 8
NSIG = 2048
SPC = NSIG // NCORES   # 256 signals per core
XCOLS = NWIN * 2 * SPC  # packed input columns = 9216
TAIL = T - (NWIN - 1) * W  # 16 real outputs in the last window

# input chunk boundaries, in windows (small early chunks so the first
# windows start while the DMA subsystem is still ramping)
CHUNKS = [(0, 1), (1, 3), (3, 6), (6, 10), (10, 14), (14, 18)]
# output chunks, in windows (small final chunks shorten the kernel tail)
OCHUNKS = [(0, 5), (5, 10), (10, 13), (13, 15), (15, 17), (17, 18)]
YCOLS = (NWIN - 1) * 2 * SPC + SPC  # 8960: block1 of the last window unused


# ----------------------------------------------------------------------------
# host-side: derive block-filter matrices from sos
# ----------------------------------------------------------------------------

def _build_system(sos):
    """Cascade of biquads (DF2T) -> single state space (A, B, C, D), float64."""
    sos = np.asarray(sos, dtype=np.float64)
    A = np.zeros((0, 0))
    B = np.zeros((0,))
    C = np.zeros((0,))
    D = 1.0
    for (b0, b1, b2, _one, a1, a2) in sos:
        As = np.array([[-a1, 1.0], [-a2, 0.0]])
        Bs = np.array([b1 - a1 * b0, b2 - a2 * b0])
        Cs = np.array([1.0, 0.0])
        Ds = b0
        n = A.shape[0]
        Anew = np.zeros((n + 2, n + 2))
        Anew[:n, :n] = A
        Anew[n:, :n] = np.outer(Bs, C)
        Anew[n:, n:] = As
        A = Anew
        B = np.concatenate([B, Bs * D])
        C = np.concatenate([Ds * C, Cs])
        D = Ds * D
    return A, B, C, D


def _balance(A, B, C):
    """Square-root balanced realization: keeps intermediate state magnitudes
    O(1) so the fp16 state rows don't lose precision."""
    Pg = np.outer(B, B)
    Ak = A.copy()
    for _ in range(64):
        Pg = Pg + Ak @ Pg @ Ak.T
        Ak = Ak @ Ak
    Q = np.outer(C, C)
    Ak = A.copy()
    for _ in range(64):
        Q = Q + Ak.T @ Q @ Ak
        Ak = Ak @ Ak
    Rc = np.linalg.cholesky(Pg + 1e-30 * np.eye(len(B)))
    M = Rc.T @ Q @ Rc
    lam, U = np.linalg.eigh(M)
    lam = np.maximum(lam, 1e-30)
    Tm = Rc @ U @ np.diag(lam ** -0.25)
    Ti = np.diag(lam ** 0.25) @ U.T @ np.linalg.inv(Rc)
    return Ti @ A @ Tm, Ti @ B, C @ Tm


def _permute_rows(m, state_rows):
    """[120, n] tau-major -> [128, n] with taus 96..119 at rows 104..127 and
    state_rows [8, n] at rows 96..103."""
    out = np.zeros((P, m.shape[1]))
    out[:SROW] = m[:SROW]
    out[SROW + NST:] = m[SROW:]
    out[SROW:SROW + NST] = state_rows
    return out


def _permute_cols(m, state_cols):
    """[128, 120] -> [128, 128] with output-t columns 96..119 at cols
    104..127 and state_cols [128, 8] at cols 96..103."""
    out = np.zeros((P, P))
    out[:, :SROW] = m[:, :SROW]
    out[:, SROW + NST:] = m[:, SROW:]
    out[:, SROW:SROW + NST] = state_cols
    return out


def _build_tables(sos):
    """Fused fp16 operator tables ctab[128, 384] = [L1 | L2 | L3], each
    [128, 128]: K-rows permuted like the packed input (taus + state rows at
    96..103), M-columns permuted like the psum output (yT rows + state
    columns at 96..103).
    """
    A, B, C, D = _build_system(sos)
    A, B, C = _balance(A, B, C)
    ns = A.shape[0]
    assert ns == NST

    h = np.zeros(L)
    h[0] = D
    An = np.eye(ns)
    for k in range(1, L):
        h[k] = C @ An @ B
        An = An @ A
    Th = np.zeros((L, L))          # Th[tau, t] = h[t - tau]
    for m in range(L):
        Th[m, m:] = h[: L - m]

    Z = np.zeros((L, ns))          # Z[n] = C A^n
    CAn = C.copy()
    for n in range(L):
        Z[n] = CAn
        CAn = CAn @ A

    F = np.zeros((ns, L))          # F[:, m] = A^(L-1-m) B
    AmB = B.copy()
    for m in range(L - 1, -1, -1):
        F[:, m] = AmB
        AmB = A @ AmB

    AL = np.linalg.matrix_power(A, L)

    # L1: block0 outputs from b0 -- conv Th + entry-state correction Z s
    L1 = _permute_cols(_permute_rows(Th, Z.T), np.zeros((P, NST)))
    # L2: block1 outputs + state update, from b0 -- cross-block (Z F),
    # state correction (Z A_L), state update (A_L F | A_L^2)
    L2 = _permute_cols(_permute_rows((Z @ F).T, (Z @ AL).T),
                       _permute_rows((AL @ F).T, (AL @ AL).T))
    # L3: block1 outputs + state update, from b1 -- conv Th + F x1
    L3 = _permute_cols(_permute_rows(Th, np.zeros((NST, L))),
                       _permute_rows(F.T, np.zeros((NST, NST))))

    ctab = np.concatenate([L1, L2, L3], axis=1)
    return np.ascontiguousarray(ctab, dtype=np.float16)


def _pack_input(xc):
    """xc [SPC, T] fp32 -> packed [128, XCOLS] fp16 with the tau-row
    permutation of _permute_rows; rows 96..103 zero (these carry the zero
    initial state for window 0, and stay zero for block-1 columns)."""
    pad = np.zeros((SPC, TPAD), np.float32)
    pad[:, :T] = xc
    blocks = pad.reshape(SPC, NWIN * 2, L)         # [sig, blk, tau]
    arr = blocks.transpose(2, 1, 0).reshape(L, XCOLS)
    out = np.zeros((P, XCOLS), np.float16)
    out[:SROW] = arr[:SROW]
    out[SROW + NST:] = arr[SROW:]
    return out


def _unpack_output(y16):
    """y16 [128, XCOLS] fp16 (packed yT, permuted rows) -> [SPC, T] fp32."""
    arr = np.concatenate(
        [y16[:SROW], y16[SROW + NST:]], axis=0).astype(np.float32)
    blocks = arr.reshape(L, NWIN * 2, SPC)          # [t, blk, sig]
    return blocks.transpose(2, 1, 0).reshape(SPC, TPAD)[:, :T]


# ----------------------------------------------------------------------------
# device kernel
# ----------------------------------------------------------------------------

def _build_nc():
    nc = bacc.Bacc("TRN2", target_bir_lowering=False)
    xp_d = nc.dram_tensor("xp", [P, XCOLS], FP16, kind="ExternalInput").ap()
    ctab_d = nc.dram_tensor("ctab", [P, 3 * P], FP16,
                            kind="ExternalInput").ap()
    y_d = nc.dram_tensor("y", [P, XCOLS], FP16, kind="ExternalOutput").ap()

    with tile.TileContext(nc) as tc:
        with (
            tc.tile_pool(name="consts", bufs=1) as consts,
            tc.tile_pool(name="xpool", bufs=4) as xpool,
            tc.tile_pool(name="ypool", bufs=3) as ypool,
            tc.tile_pool(name="py", bufs=4, space="PSUM") as pyp,
            tc.tile_pool(name="pb", bufs=4, space="PSUM") as psb,
        ):
            # first x chunk leads the SP DMA queue; constants go on the
            # scalar-engine queue so they don't delay it
            nw0 = CHUNKS[0][1]
            xw0 = xpool.tile([P, nw0 * 2 * SPC], FP16, tag="xc0")
            nc.sync.dma_start(xw0, xp_d[:, 0:nw0 * 2 * SPC])
            ctab = consts.tile([P, 3 * P], FP16)
            nc.scalar.dma_start(ctab, ctab_d)
            l1 = ctab[:, 0:P]
            l2 = ctab[:, P:2 * P]
            l3 = ctab[:, 2 * P:3 * P]

            # PE clock warmup while the first chunk's DMA is in flight
            scr_a = consts.tile([P, P], FP16, tag="scr_a")
            scr_b = consts.tile([P, P], FP16, tag="scr_b")
            nc.vector.memset(scr_a, 0.0)
            nc.vector.memset(scr_b, 0.0)
            pwu = pyp.tile([P, P], FP32, tag="py", name="pwu")
            for _ in range(20):
                nc.tensor.matmul(pwu, scr_a, scr_b, start=True, stop=True)

            # chunk tiles: [128, n_windows*512] fp16, col w*512+b*256+sig
            xtiles = {}          # window -> (tile, col offset of window)
            for (w0, w1) in CHUNKS:
                if w0 == 0:
                    ct = xw0
                else:
                    ct = xpool.tile([P, (w1 - w0) * 2 * SPC], FP16, tag="xc")
                    nc.sync.dma_start(
                        ct, xp_d[:, w0 * 2 * SPC:w1 * 2 * SPC])
                for w in range(w0, w1):
                    xtiles[w] = (ct, (w - w0) * 2 * SPC)

            ybuf = None
            yoff0 = 0
            for w in range(NWIN):
                xt, c0 = xtiles[w]
                b0 = xt[:, c0:c0 + SPC]              # block0 + state rows
                b1 = xt[:, c0 + SPC:c0 + 2 * SPC]    # block1 (state rows zero)

                for (ow0, ow1) in OCHUNKS:
                    if w == ow0:
                        ybuf = ypool.tile([P, 5 * 2 * SPC], FP16, tag="yb",
                                          name=f"yb_{w}")
                        yoff0 = ow0
                yoff = (w - yoff0) * 2 * SPC

                # psum_a: block0 outputs (yT); psum_b: block1 outputs with
                # the next state in rows 96..103 (separate banks so the two
                # accumulation groups don't interleave within one bank)
                psum_a = pyp.tile([P, SPC], FP32, tag="py", name=f"pya_{w}")
                last = w == NWIN - 1
                # mm order: mm3 (state-free filler), mm2 (finalizes state),
                # mm1 LAST -- so the critical state copy becomes ready
                # before psum_a's y-copy on the DVE FIFO
                if not last:
                    psum_b = psb.tile([P, SPC], FP32, tag="pb",
                                      name=f"pyb_{w}")
                    nc.tensor.matmul(psum_b, l3, b1, start=True, stop=False)
                    nc.tensor.matmul(psum_b, l2, b0, start=False, stop=True)
                nc.tensor.matmul(psum_a, l1, b0, start=True, stop=True)

                if not last:
                    # copy next-window entry state (psum_b rows 96..103)
                    # into the next b0 tile's spare rows -- critical chain,
                    # single fast DVE copy
                    nxt, nc0 = xtiles[w + 1]
                    srows = slice(SROW, SROW + NST)
                    nc.vector.tensor_copy(
                        nxt[srows, nc0:nc0 + SPC], psum_b[srows, :])

                # stage yT to SBUF as fp16, balanced across DVE and ACT
                # (DVE also carries the critical state copy)
                nc.vector.tensor_copy(
                    ybuf[:, yoff:yoff + P], psum_a[:, 0:P])
                nc.scalar.copy(
                    ybuf[:, yoff + P:yoff + SPC], psum_a[:, P:SPC])
                if not last:
                    nc.scalar.copy(
                        ybuf[:, yoff + SPC:yoff + 2 * SPC], psum_b)

                # flush output chunk on the sync queue
                for (ow0, ow1) in OCHUNKS:
                    if w == ow1 - 1:
                        cstart = ow0 * 2 * SPC
                        cend = min(ow1 * 2 * SPC, YCOLS)
                        nc.sync.dma_start(
                            y_d[:, cstart:cend],
                            ybuf[:, 0:cend - cstart])
    nc.compile()
    return nc


_NC_CACHE = None
LAST_RESULTS = None  # BassKernelResults of the most recent kernel() call


def _get_nc():
    global _NC_CACHE
    if _NC_CACHE is None:
        _NC_CACHE = _build_nc()
    return _NC_CACHE


def kernel(x: np.ndarray, sos: np.ndarray) -> np.ndarray:
    x = np.asarray(x)
    orig_shape = x.shape
    orig_dtype = x.dtype
    ctab = _build_tables(np.asarray(sos, dtype=np.float64))

    xf = np.ascontiguousarray(x.reshape(NSIG, T), dtype=np.float32)
    in_maps = [
        {"xp": _pack_input(xf[c * SPC:(c + 1) * SPC]), "ctab": ctab}
        for c in range(NCORES)
    ]
    nc = _get_nc()
    res = run_bass_kernel_spmd(nc, in_maps, core_ids=list(range(NCORES)))
    global LAST_RESULTS
    LAST_RESULTS = res
    y = np.concatenate(
        [_unpack_output(res.results[c]["y"]) for c in range(NCORES)], axis=0)
    return y.reshape(orig_shape).astype(orig_dtype, copy=False)


# revision 51
# speedup vs baseline: 1.0252x; 1.0252x over previous
"""Butterworth bandpass (cascaded biquad IIR) Trainium2 kernel.

Problem: y = sosfilt(sos, x) over x[32, 64, 4096] fp32 -- 2048 independent
signals, 4 cascaded DF2T biquads, sequential over T=4096.

Strategy (exact block-parallel reformulation):
  The cascade is a linear state-space system (A[8,8], B, C, D).  Split T
  into blocks of L=120 steps, two blocks per window (W=240).  The input is
  pre-transposed and fp16-packed on the HOST into [tau, block, signal]
  layout, so no PE transposes are needed, and the 8-dim state s_w at each
  window entry is carried in the 8 spare partition rows (120..127) of the
  block-0 operand tile.  All filter operators are folded into two fp16
  tables built on host in float64:

    T0[128, 240]: rows 0..119 = [Th | (Z F)^T]   (conv + cross-block)
                  rows 120..  = [Z^T | (Z A_L)^T] (entry-state correction)
    G0[128, 8]:   rows 0..119 = (A_L F)^T, rows 120.. = (A_L^2)^T
    G1[120, 8]:   F^T

  Per window only 3 matmuls (all operands fp16, psum fp32), computing the
  TRANSPOSED output yT[t, sig] so the coefficient tables are the stationary
  operand and each matmul streams all 256 signals (N=256):
    psum[128, 512]: cols 0:256 = block0 outputs, cols 256:512 = block1
    mm3: psum[:,256:512]  = L3.T @ b1   (block1 conv + F x1 state part)
    mm1: psum[:,0:256]    = L1.T @ b0   (block0 conv + Z s correction)
    mm2: psum[:,256:512] += L2.T @ b0   (cross-block + Z A_L s + state upd)
  The 8-dim state rides in 8 spare K-rows of b0 and 8 spare output rows of
  the block1 columns, so corrections and the state update are free.  mm3 is
  state-independent and ordered first: it fills the PE while the previous
  window's state copy is in flight.  Engine accesses must start at a
  32-aligned partition, so state rows live at partitions 96..103 (input
  taus 96..119 shift to rows 104..127); tables are permuted to match in
  rows (K) and columns (psum partitions).  y returns to HBM fp16 in the
  same packed [row, block*256+sig] layout as the input (un-packed on the
  host); 2048 signals are sharded 256 per NeuronCore.
"""

import numpy as np

import concourse.bass as bass
import concourse.tile as tile
from concourse import bacc
from concourse import mybir
from concourse.bass_utils import run_bass_kernel_spmd

FP32 = mybir.dt.float32
FP16 = mybir.dt.float16

P = 128            # partition width
L = 120            # time-block length (128 - 8 state rows)
SROW = 96          # partition row where the 8 state rows live (32-aligned)
NST = 8            # state dim of the 4-biquad cascade
R = 2              # blocks per window
W = R * L          # 240 time steps per window
T = 4096
NWIN = 18          # 18 windows cover 4320 >= 4096 (last window: 16 real steps)
TPAD = NWIN * W    # 4320
NCORES = 8
NSIG = 2048
SPC = NSIG // NCORES   # 256 signals per core
XCOLS = NWIN * 2 * SPC  # packed input columns = 9216
TAIL = T - (NWIN - 1) * W  # 16 real outputs in the last window

# input chunk boundaries, in windows (small early chunks so the first
# windows start while the DMA subsystem is still ramping)
CHUNKS = [(0, 1), (1, 3), (3, 6), (6, 10), (10, 14), (14, 18)]
# output chunks, in windows (small final chunks shorten the kernel tail)
OCHUNKS = [(0, 5), (5, 10), (10, 13), (13, 15), (15, 17), (17, 18)]
YCOLS = (NWIN - 1) * 2 * SPC + SPC  # 8960: block1 of the last window unused


# ----------------------------------------------------------------------------
# host-side: derive block-filter matrices from sos
# ----------------------------------------------------------------------------

def _build_system(sos):
    """Cascade of biquads (DF2T) -> single state space (A, B, C, D), float64."""
    sos = np.asarray(sos, dtype=np.float64)
    A = np.zeros((0, 0))
    B = np.zeros((0,))
    C = np.zeros((0,))
    D = 1.0
    for (b0, b1, b2, _one, a1, a2) in sos:
        As = np.array([[-a1, 1.0], [-a2, 0.0]])
        Bs = np.array([b1 - a1 * b0, b2 - a2 * b0])
        Cs = np.array([1.0, 0.0])
        Ds = b0
        n = A.shape[0]
        Anew = np.zeros((n + 2, n + 2))
        Anew[:n, :n] = A
        Anew[n:, :n] = np.outer(Bs, C)
        Anew[n:, n:] = As
        A = Anew
        B = np.concatenate([B, Bs * D])
        C = np.concatenate([Ds * C, Cs])
        D = Ds * D
    return A, B, C, D


def _balance(A, B, C):
    """Square-root balanced realization: keeps intermediate state magnitudes
    O(1) so the fp16 state rows don't lose precision."""
    Pg = np.outer(B, B)
    Ak = A.copy()
    for _ in range(64):
        Pg = Pg + Ak @ Pg @ Ak.T
        Ak = Ak @ Ak
    Q = np.outer(C, C)
    Ak = A.copy()
    for _ in range(64):
        Q = Q + Ak.T @ Q @ Ak
        Ak = Ak @ Ak
    Rc = np.linalg.cholesky(Pg + 1e-30 * np.eye(len(B)))
    M = Rc.T @ Q @ Rc
    lam, U = np.linalg.eigh(M)
    lam = np.maximum(lam, 1e-30)
    Tm = Rc @ U @ np.diag(lam ** -0.25)
    Ti = np.diag(lam ** 0.25) @ U.T @ np.linalg.inv(Rc)
    return Ti @ A @ Tm, Ti @ B, C @ Tm


def _permute_rows(m, state_rows):
    """[120, n] tau-major -> [128, n] with taus 96..119 at rows 104..127 and
    state_rows [8, n] at rows 96..103."""
    out = np.zeros((P, m.shape[1]))
    out[:SROW] = m[:SROW]
    out[SROW + NST:] = m[SROW:]
    out[SROW:SROW + NST] = state_rows
    return out


def _permute_cols(m, state_cols):
    """[128, 120] -> [128, 128] with output-t columns 96..119 at cols
    104..127 and state_cols [128, 8] at cols 96..103."""
    out = np.zeros((P, P))
    out[:, :SROW] = m[:, :SROW]
    out[:, SROW + NST:] = m[:, SROW:]
    out[:, SROW:SROW + NST] = state_cols
    return out


def _build_tables(sos):
    """Fused fp16 operator tables ctab[128, 384] = [L1 | L2 | L3], each
    [128, 128]: K-rows permuted like the packed input (taus + state rows at
    96..103), M-columns permuted like the psum output (yT rows + state
    columns at 96..103).
    """
    A, B, C, D = _build_system(sos)
    A, B, C = _balance(A, B, C)
    ns = A.shape[0]
    assert ns == NST

    h = np.zeros(L)
    h[0] = D
    An = np.eye(ns)
    for k in range(1, L):
        h[k] = C @ An @ B
        An = An @ A
    Th = np.zeros((L, L))          # Th[tau, t] = h[t - tau]
    for m in range(L):
        Th[m, m:] = h[: L - m]

    Z = np.zeros((L, ns))          # Z[n] = C A^n
    CAn = C.copy()
    for n in range(L):
        Z[n] = CAn
        CAn = CAn @ A

    F = np.zeros((ns, L))          # F[:, m] = A^(L-1-m) B
    AmB = B.copy()
    for m in range(L - 1, -1, -1):
        F[:, m] = AmB
        AmB = A @ AmB

    AL = np.linalg.matrix_power(A, L)

    # L1: block0 outputs from b0 -- conv Th + entry-state correction Z s
    L1 = _permute_cols(_permute_rows(Th, Z.T), np.zeros((P, NST)))
    # L2: block1 outputs + state update, from b0 -- cross-block (Z F),
    # state correction (Z A_L), state update (A_L F | A_L^2)
    L2 = _permute_cols(_permute_rows((Z @ F).T, (Z @ AL).T),
                       _permute_rows((AL @ F).T, (AL @ AL).T))
    # L3: block1 outputs + state update, from b1 -- conv Th + F x1
    L3 = _permute_cols(_permute_rows(Th, np.zeros((NST, L))),
                       _permute_rows(F.T, np.zeros((NST, NST))))

    ctab = np.concatenate([L1, L2, L3], axis=1)
    return np.ascontiguousarray(ctab, dtype=np.float16)


def _pack_input(xc):
    """xc [SPC, T] fp32 -> packed [128, XCOLS] fp16 with the tau-row
    permutation of _permute_rows; rows 96..103 zero (these carry the zero
    initial state for window 0, and stay zero for block-1 columns)."""
    pad = np.zeros((SPC, TPAD), np.float32)
    pad[:, :T] = xc
    blocks = pad.reshape(SPC, NWIN * 2, L)         # [sig, blk, tau]
    arr = blocks.transpose(2, 1, 0).reshape(L, XCOLS)
    out = np.zeros((P, XCOLS), np.float16)
    out[:SROW] = arr[:SROW]
    out[SROW + NST:] = arr[SROW:]
    return out


def _unpack_output(y16):
    """y16 [128, XCOLS] fp16 (packed yT, permuted rows) -> [SPC, T] fp32."""
    arr = np.concatenate(
        [y16[:SROW], y16[SROW + NST:]], axis=0).astype(np.float32)
    blocks = arr.reshape(L, NWIN * 2, SPC)          # [t, blk, sig]
    return blocks.transpose(2, 1, 0).reshape(SPC, TPAD)[:, :T]


# ----------------------------------------------------------------------------
# device kernel
# ----------------------------------------------------------------------------

def _build_nc():
    nc = bacc.Bacc("TRN2", target_bir_lowering=False)
    xp_d = nc.dram_tensor("xp", [P, XCOLS], FP16, kind="ExternalInput").ap()
    ctab_d = nc.dram_tensor("ctab", [P, 3 * P], FP16,
                            kind="ExternalInput").ap()
    y_d = nc.dram_tensor("y", [P, XCOLS], FP16, kind="ExternalOutput").ap()

    with tile.TileContext(nc) as tc:
        with (
            tc.tile_pool(name="consts", bufs=1) as consts,
            tc.tile_pool(name="xpool", bufs=3) as xpool,
            tc.tile_pool(name="ypool", bufs=3) as ypool,
            tc.tile_pool(name="py", bufs=4, space="PSUM") as pyp,
            tc.tile_pool(name="pb", bufs=4, space="PSUM") as psb,
        ):
            # first x chunk leads the SP DMA queue; constants go on the
            # scalar-engine queue so they don't delay it
            nw0 = CHUNKS[0][1]
            xw0 = xpool.tile([P, nw0 * 2 * SPC], FP16, tag="xc0")
            nc.sync.dma_start(xw0, xp_d[:, 0:nw0 * 2 * SPC])
            ctab = consts.tile([P, 3 * P], FP16)
            nc.scalar.dma_start(ctab, ctab_d)
            l1 = ctab[:, 0:P]
            l2 = ctab[:, P:2 * P]
            l3 = ctab[:, 2 * P:3 * P]

            # PE clock warmup while the first chunk's DMA is in flight
            scr_a = consts.tile([P, P], FP16, tag="scr_a")
            scr_b = consts.tile([P, P], FP16, tag="scr_b")
            nc.vector.memset(scr_a, 0.0)
            nc.vector.memset(scr_b, 0.0)
            pwu = pyp.tile([P, P], FP32, tag="py", name="pwu")
            for _ in range(20):
                nc.tensor.matmul(pwu, scr_a, scr_b, start=True, stop=True)

            # chunk tiles: [128, n_windows*512] fp16, col w*512+b*256+sig
            xtiles = {}          # window -> (tile, col offset of window)
            for (w0, w1) in CHUNKS:
                if w0 == 0:
                    ct = xw0
                else:
                    ct = xpool.tile([P, (w1 - w0) * 2 * SPC], FP16, tag="xc")
                    nc.sync.dma_start(
                        ct, xp_d[:, w0 * 2 * SPC:w1 * 2 * SPC])
                for w in range(w0, w1):
                    xtiles[w] = (ct, (w - w0) * 2 * SPC)

            ybuf = None
            yoff0 = 0
            for w in range(NWIN):
                xt, c0 = xtiles[w]
                b0 = xt[:, c0:c0 + SPC]              # block0 + state rows
                b1 = xt[:, c0 + SPC:c0 + 2 * SPC]    # block1 (state rows zero)

                for (ow0, ow1) in OCHUNKS:
                    if w == ow0:
                        ybuf = ypool.tile([P, 5 * 2 * SPC], FP16, tag="yb",
                                          name=f"yb_{w}")
                        yoff0 = ow0
                yoff = (w - yoff0) * 2 * SPC

                # psum_a: block0 outputs (yT); psum_b: block1 outputs with
                # the next state in rows 96..103 (separate banks so the two
                # accumulation groups don't interleave within one bank)
                psum_a = pyp.tile([P, SPC], FP32, tag="py", name=f"pya_{w}")
                last = w == NWIN - 1
                # mm order: mm3 (state-free filler), mm2 (finalizes state),
                # mm1 LAST -- so the critical state copy becomes ready
                # before psum_a's y-copy on the DVE FIFO
                if not last:
                    psum_b = psb.tile([P, SPC], FP32, tag="pb",
                                      name=f"pyb_{w}")
                    nc.tensor.matmul(psum_b, l3, b1, start=True, stop=False)
                    nc.tensor.matmul(psum_b, l2, b0, start=False, stop=True)
                nc.tensor.matmul(psum_a, l1, b0, start=True, stop=True)

                if not last:
                    # copy next-window entry state (psum_b rows 96..103)
                    # into the next b0 tile's spare rows -- critical chain,
                    # single fast DVE copy
                    nxt, nc0 = xtiles[w + 1]
                    srows = slice(SROW, SROW + NST)
                    nc.vector.tensor_copy(
                        nxt[srows, nc0:nc0 + SPC], psum_b[srows, :])

                # stage yT to SBUF as fp16, balanced across DVE and ACT
                # (DVE also carries the critical state copy)
                nc.vector.tensor_copy(
                    ybuf[:, yoff:yoff + P], psum_a[:, 0:P])
                nc.scalar.copy(
                    ybuf[:, yoff + P:yoff + SPC], psum_a[:, P:SPC])
                if not last:
                    nc.scalar.copy(
                        ybuf[:, yoff + SPC:yoff + 2 * SPC], psum_b)

                # flush output chunk on the sync queue
                for (ow0, ow1) in OCHUNKS:
                    if w == ow1 - 1:
                        cstart = ow0 * 2 * SPC
                        cend = min(ow1 * 2 * SPC, YCOLS)
                        nc.sync.dma_start(
                            y_d[:, cstart:cend],
                            ybuf[:, 0:cend - cstart])
    nc.compile()
    return nc


_NC_CACHE = None
LAST_RESULTS = None  # BassKernelResults of the most recent kernel() call


def _get_nc():
    global _NC_CACHE
    if _NC_CACHE is None:
        _NC_CACHE = _build_nc()
    return _NC_CACHE


def kernel(x: np.ndarray, sos: np.ndarray) -> np.ndarray:
    x = np.asarray(x)
    orig_shape = x.shape
    orig_dtype = x.dtype
    ctab = _build_tables(np.asarray(sos, dtype=np.float64))

    xf = np.ascontiguousarray(x.reshape(NSIG, T), dtype=np.float32)
    in_maps = [
        {"xp": _pack_input(xf[c * SPC:(c + 1) * SPC]), "ctab": ctab}
        for c in range(NCORES)
    ]
    nc = _get_nc()
    res = run_bass_kernel_spmd(nc, in_maps, core_ids=list(range(NCORES)))
    global LAST_RESULTS
    LAST_RESULTS = res
    y = np.concatenate(
        [_unpack_output(res.results[c]["y"]) for c in range(NCORES)], axis=0)
    return y.reshape(orig_shape).astype(orig_dtype, copy=False)


# revision 52
# speedup vs baseline: 1.1083x; 1.0811x over previous
"""Butterworth bandpass (cascaded biquad IIR) Trainium2 kernel.

Problem: y = sosfilt(sos, x) over x[32, 64, 4096] fp32 -- 2048 independent
signals, 4 cascaded DF2T biquads, sequential over T=4096.

Strategy (exact block-parallel reformulation):
  The cascade is a linear state-space system (A[8,8], B, C, D).  Split T
  into blocks of L=120 steps, two blocks per window (W=240).  The input is
  pre-transposed and fp16-packed on the HOST into [tau, block, signal]
  layout, so no PE transposes are needed, and the 8-dim state s_w at each
  window entry is carried in the 8 spare partition rows (120..127) of the
  block-0 operand tile.  All filter operators are folded into two fp16
  tables built on host in float64:

    T0[128, 240]: rows 0..119 = [Th | (Z F)^T]   (conv + cross-block)
                  rows 120..  = [Z^T | (Z A_L)^T] (entry-state correction)
    G0[128, 8]:   rows 0..119 = (A_L F)^T, rows 120.. = (A_L^2)^T
    G1[120, 8]:   F^T

  Per window only 3 matmuls (all operands fp16, psum fp32), computing the
  TRANSPOSED output yT[t, sig] so the coefficient tables are the stationary
  operand and each matmul streams all 256 signals (N=256):
    psum[128, 512]: cols 0:256 = block0 outputs, cols 256:512 = block1
    mm3: psum[:,256:512]  = L3.T @ b1   (block1 conv + F x1 state part)
    mm1: psum[:,0:256]    = L1.T @ b0   (block0 conv + Z s correction)
    mm2: psum[:,256:512] += L2.T @ b0   (cross-block + Z A_L s + state upd)
  The 8-dim state rides in 8 spare K-rows of b0 and 8 spare output rows of
  the block1 columns, so corrections and the state update are free.  mm3 is
  state-independent and ordered first: it fills the PE while the previous
  window's state copy is in flight.  Engine accesses must start at a
  32-aligned partition, so state rows live at partitions 96..103 (input
  taus 96..119 shift to rows 104..127); tables are permuted to match in
  rows (K) and columns (psum partitions).  y returns to HBM fp16 in the
  same packed [row, block*256+sig] layout as the input (un-packed on the
  host); 2048 signals are sharded 256 per NeuronCore.
"""

import numpy as np

import concourse.bass as bass
import concourse.tile as tile
from concourse import bacc
from concourse import mybir
from concourse.bass_utils import run_bass_kernel_spmd

FP32 = mybir.dt.float32
FP16 = mybir.dt.float16

P = 128            # partition width
L = 120            # time-block length (128 - 8 state rows)
SROW = 96          # partition row where the 8 state rows live (32-aligned)
NST = 8            # state dim of the 4-biquad cascade
R = 2              # blocks per window
W = R * L          # 240 time steps per window
T = 4096
NWIN = 18          # 18 windows cover 4320 >= 4096 (last window: 16 real steps)
TPAD = NWIN * W    # 4320
NCORES = 8
NSIG = 2048
SPC = NSIG // NCORES   # 256 signals per core
XCOLS = NWIN * 2 * SPC  # packed input columns = 9216
TAIL = T - (NWIN - 1) * W  # 16 real outputs in the last window

# input chunk boundaries, in windows (small early chunks so the first
# windows start while the DMA subsystem is still ramping)
CHUNKS = [(0, 1), (1, 3), (3, 6), (6, 10), (10, 14), (14, 18)]
# output chunks, in windows (small final chunks shorten the kernel tail)
OCHUNKS = [(0, 5), (5, 10), (10, 13), (13, 15), (15, 17), (17, 18)]
YCOLS = (NWIN - 1) * 2 * SPC + SPC  # 8960: block1 of the last window unused


# ----------------------------------------------------------------------------
# host-side: derive block-filter matrices from sos
# ----------------------------------------------------------------------------

def _build_system(sos):
    """Cascade of biquads (DF2T) -> single state space (A, B, C, D), float64."""
    sos = np.asarray(sos, dtype=np.float64)
    A = np.zeros((0, 0))
    B = np.zeros((0,))
    C = np.zeros((0,))
    D = 1.0
    for (b0, b1, b2, _one, a1, a2) in sos:
        As = np.array([[-a1, 1.0], [-a2, 0.0]])
        Bs = np.array([b1 - a1 * b0, b2 - a2 * b0])
        Cs = np.array([1.0, 0.0])
        Ds = b0
        n = A.shape[0]
        Anew = np.zeros((n + 2, n + 2))
        Anew[:n, :n] = A
        Anew[n:, :n] = np.outer(Bs, C)
        Anew[n:, n:] = As
        A = Anew
        B = np.concatenate([B, Bs * D])
        C = np.concatenate([Ds * C, Cs])
        D = Ds * D
    return A, B, C, D


def _balance(A, B, C):
    """Square-root balanced realization: keeps intermediate state magnitudes
    O(1) so the fp16 state rows don't lose precision."""
    Pg = np.outer(B, B)
    Ak = A.copy()
    for _ in range(64):
        Pg = Pg + Ak @ Pg @ Ak.T
        Ak = Ak @ Ak
    Q = np.outer(C, C)
    Ak = A.copy()
    for _ in range(64):
        Q = Q + Ak.T @ Q @ Ak
        Ak = Ak @ Ak
    Rc = np.linalg.cholesky(Pg + 1e-30 * np.eye(len(B)))
    M = Rc.T @ Q @ Rc
    lam, U = np.linalg.eigh(M)
    lam = np.maximum(lam, 1e-30)
    Tm = Rc @ U @ np.diag(lam ** -0.25)
    Ti = np.diag(lam ** 0.25) @ U.T @ np.linalg.inv(Rc)
    return Ti @ A @ Tm, Ti @ B, C @ Tm


def _permute_rows(m, state_rows):
    """[120, n] tau-major -> [128, n] with taus 96..119 at rows 104..127 and
    state_rows [8, n] at rows 96..103."""
    out = np.zeros((P, m.shape[1]))
    out[:SROW] = m[:SROW]
    out[SROW + NST:] = m[SROW:]
    out[SROW:SROW + NST] = state_rows
    return out


def _permute_cols(m, state_cols):
    """[128, 120] -> [128, 128] with output-t columns 96..119 at cols
    104..127 and state_cols [128, 8] at cols 96..103."""
    out = np.zeros((P, P))
    out[:, :SROW] = m[:, :SROW]
    out[:, SROW + NST:] = m[:, SROW:]
    out[:, SROW:SROW + NST] = state_cols
    return out


def _build_tables(sos):
    """Fused fp16 operator tables ctab[128, 384] = [L1 | L2 | L3], each
    [128, 128]: K-rows permuted like the packed input (taus + state rows at
    96..103), M-columns permuted like the psum output (yT rows + state
    columns at 96..103).
    """
    A, B, C, D = _build_system(sos)
    A, B, C = _balance(A, B, C)
    ns = A.shape[0]
    assert ns == NST

    h = np.zeros(L)
    h[0] = D
    An = np.eye(ns)
    for k in range(1, L):
        h[k] = C @ An @ B
        An = An @ A
    Th = np.zeros((L, L))          # Th[tau, t] = h[t - tau]
    for m in range(L):
        Th[m, m:] = h[: L - m]

    Z = np.zeros((L, ns))          # Z[n] = C A^n
    CAn = C.copy()
    for n in range(L):
        Z[n] = CAn
        CAn = CAn @ A

    F = np.zeros((ns, L))          # F[:, m] = A^(L-1-m) B
    AmB = B.copy()
    for m in range(L - 1, -1, -1):
        F[:, m] = AmB
        AmB = A @ AmB

    AL = np.linalg.matrix_power(A, L)

    # L1: block0 outputs from b0 -- conv Th + entry-state correction Z s
    L1 = _permute_cols(_permute_rows(Th, Z.T), np.zeros((P, NST)))
    # L2: block1 outputs + state update, from b0 -- cross-block (Z F),
    # state correction (Z A_L), state update (A_L F | A_L^2)
    L2 = _permute_cols(_permute_rows((Z @ F).T, (Z @ AL).T),
                       _permute_rows((AL @ F).T, (AL @ AL).T))
    # L3: block1 outputs + state update, from b1 -- conv Th + F x1
    L3 = _permute_cols(_permute_rows(Th, np.zeros((NST, L))),
                       _permute_rows(F.T, np.zeros((NST, NST))))

    ctab = np.concatenate([L1, L2, L3], axis=1)
    return np.ascontiguousarray(ctab, dtype=np.float16)


def _pack_input(xc):
    """xc [SPC, T] fp32 -> packed [128, XCOLS] fp16 with the tau-row
    permutation of _permute_rows; rows 96..103 zero (these carry the zero
    initial state for window 0, and stay zero for block-1 columns)."""
    pad = np.zeros((SPC, TPAD), np.float32)
    pad[:, :T] = xc
    blocks = pad.reshape(SPC, NWIN * 2, L)         # [sig, blk, tau]
    arr = blocks.transpose(2, 1, 0).reshape(L, XCOLS)
    out = np.zeros((P, XCOLS), np.float16)
    out[:SROW] = arr[:SROW]
    out[SROW + NST:] = arr[SROW:]
    return out


def _unpack_output(y16):
    """y16 [128, XCOLS] fp16 (packed yT, permuted rows) -> [SPC, T] fp32."""
    arr = np.concatenate(
        [y16[:SROW], y16[SROW + NST:]], axis=0).astype(np.float32)
    blocks = arr.reshape(L, NWIN * 2, SPC)          # [t, blk, sig]
    return blocks.transpose(2, 1, 0).reshape(SPC, TPAD)[:, :T]


# ----------------------------------------------------------------------------
# device kernel
# ----------------------------------------------------------------------------

def _build_nc():
    nc = bacc.Bacc("TRN2", target_bir_lowering=False)
    xp_d = nc.dram_tensor("xp", [P, XCOLS], FP16, kind="ExternalInput").ap()
    ctab_d = nc.dram_tensor("ctab", [P, 3 * P], FP16,
                            kind="ExternalInput").ap()
    y_d = nc.dram_tensor("y", [P, XCOLS], FP16, kind="ExternalOutput").ap()

    with tile.TileContext(nc) as tc:
        with (
            tc.tile_pool(name="consts", bufs=1) as consts,
            tc.tile_pool(name="xpool", bufs=3) as xpool,
            tc.tile_pool(name="ypool", bufs=3) as ypool,
            tc.tile_pool(name="py", bufs=4, space="PSUM") as pyp,
            tc.tile_pool(name="pb", bufs=4, space="PSUM") as psb,
        ):
            # first x chunk leads the SP DMA queue; constants go on the
            # scalar-engine queue so they don't delay it
            nw0 = CHUNKS[0][1]
            xw0 = xpool.tile([P, nw0 * 2 * SPC], FP16, tag="xc0")
            nc.sync.dma_start(xw0, xp_d[:, 0:nw0 * 2 * SPC])
            ctab = consts.tile([P, 3 * P], FP16)
            nc.scalar.dma_start(ctab, ctab_d)
            l1 = ctab[:, 0:P]
            l2 = ctab[:, P:2 * P]
            l3 = ctab[:, 2 * P:3 * P]

            # PE clock warmup while the first chunk's DMA is in flight
            scr_a = consts.tile([P, P], FP16, tag="scr_a")
            scr_b = consts.tile([P, P], FP16, tag="scr_b")
            nc.vector.memset(scr_a, 0.0)
            nc.vector.memset(scr_b, 0.0)
            pwu = pyp.tile([P, P], FP32, tag="py", name="pwu")
            for _ in range(20):
                nc.tensor.matmul(pwu, scr_a, scr_b, start=True, stop=True)

            # chunk tiles: [128, n_windows*512] fp16, col w*512+b*256+sig
            xtiles = {}          # window -> (tile, col offset of window)
            for (w0, w1) in CHUNKS:
                if w0 == 0:
                    ct = xw0
                else:
                    ct = xpool.tile([P, (w1 - w0) * 2 * SPC], FP16, tag="xc")
                    nc.sync.dma_start(
                        ct, xp_d[:, w0 * 2 * SPC:w1 * 2 * SPC])
                for w in range(w0, w1):
                    xtiles[w] = (ct, (w - w0) * 2 * SPC)

            ybuf = None
            yoff0 = 0
            for w in range(NWIN):
                xt, c0 = xtiles[w]
                b0 = xt[:, c0:c0 + SPC]              # block0 + state rows
                b1 = xt[:, c0 + SPC:c0 + 2 * SPC]    # block1 (state rows zero)

                for (ow0, ow1) in OCHUNKS:
                    if w == ow0:
                        ybuf = ypool.tile([P, 5 * 2 * SPC], FP16, tag="yb",
                                          name=f"yb_{w}")
                        yoff0 = ow0
                yoff = (w - yoff0) * 2 * SPC

                # psum_a: block0 outputs (yT); psum_b: block1 outputs with
                # the next state in rows 96..103 (separate banks so the two
                # accumulation groups don't interleave within one bank)
                psum_a = pyp.tile([P, SPC], FP32, tag="py", name=f"pya_{w}")
                last = w == NWIN - 1
                # mm order: mm3 (state-free filler), mm2 (finalizes state),
                # mm1 LAST -- so the critical state copy becomes ready
                # before psum_a's y-copy on the DVE FIFO
                if not last:
                    psum_b = psb.tile([P, SPC], FP32, tag="pb",
                                      name=f"pyb_{w}")
                    nc.tensor.matmul(psum_b, l3, b1, start=True, stop=False)
                    nc.tensor.matmul(psum_b, l2, b0, start=False, stop=True)
                nc.tensor.matmul(psum_a, l1, b0, start=True, stop=True)

                if not last:
                    # copy next-window entry state (psum_b rows 96..103)
                    # into the next b0 tile's spare rows -- critical chain,
                    # single fast DVE copy
                    nxt, nc0 = xtiles[w + 1]
                    srows = slice(SROW, SROW + NST)
                    nc.vector.tensor_copy(
                        nxt[srows, nc0:nc0 + SPC], psum_b[srows, :])

                # stage yT to SBUF as fp16 on ACT only, so the DVE is
                # dedicated to the critical-chain state copies
                nc.scalar.copy(ybuf[:, yoff:yoff + SPC], psum_a)
                if not last:
                    nc.scalar.copy(
                        ybuf[:, yoff + SPC:yoff + 2 * SPC], psum_b)

                # flush output chunk on the sync queue
                for (ow0, ow1) in OCHUNKS:
                    if w == ow1 - 1:
                        cstart = ow0 * 2 * SPC
                        cend = min(ow1 * 2 * SPC, YCOLS)
                        nc.sync.dma_start(
                            y_d[:, cstart:cend],
                            ybuf[:, 0:cend - cstart])
    nc.compile()
    return nc


_NC_CACHE = None
LAST_RESULTS = None  # BassKernelResults of the most recent kernel() call


def _get_nc():
    global _NC_CACHE
    if _NC_CACHE is None:
        _NC_CACHE = _build_nc()
    return _NC_CACHE


def kernel(x: np.ndarray, sos: np.ndarray) -> np.ndarray:
    x = np.asarray(x)
    orig_shape = x.shape
    orig_dtype = x.dtype
    ctab = _build_tables(np.asarray(sos, dtype=np.float64))

    xf = np.ascontiguousarray(x.reshape(NSIG, T), dtype=np.float32)
    in_maps = [
        {"xp": _pack_input(xf[c * SPC:(c + 1) * SPC]), "ctab": ctab}
        for c in range(NCORES)
    ]
    nc = _get_nc()
    res = run_bass_kernel_spmd(nc, in_maps, core_ids=list(range(NCORES)))
    global LAST_RESULTS
    LAST_RESULTS = res
    y = np.concatenate(
        [_unpack_output(res.results[c]["y"]) for c in range(NCORES)], axis=0)
    return y.reshape(orig_shape).astype(orig_dtype, copy=False)
